# revision 1
# baseline (speedup 1.0000x reference)
"""2-layer GCN (GCNConv -> relu -> GCNConv -> mean) on 8 trn2 NeuronCores.

Math restructure:
  reference output = mean_n(h2[n]) with h2 = A_norm @ (h1 @ W2) + b2,
  h1 = relu(A_norm @ (x @ W1) + b1), A_norm = D^-1/2 (A+I) D^-1/2.
  Since mean is linear and every edge lands on exactly one dst:
    mean(h2) = (1/N) * (sum_n w_n * h1[n]) @ W2 + b2,
    w_n = dis_n * sum_{e: src_e = n} dis_{dst_e}   (edges incl. self-loops)
  so only layer 1 needs real message passing.

Device work per core (SPMD, same program, different data):
  phase 1: h' = (dis * x) @ W1 for ALL nodes (replicated; avoids collectives),
           stored to local DRAM as fp16 rows.
  phase 2: for each of this core's 49 dst-tiles (128 nodes each):
           one big indirect-DMA gather of all edge source rows, then per
           128-edge K-tile a one-hot selection matrix S (is_equal vs iota)
           and a PE matmul S^T @ msgs accumulating [128 dst x 128 feat] in
           PSUM; epilogue: *dis_dst, +b1, relu, then w-weighted reduction
           matmul into a persistent [1,128] PSUM accumulator.
  host:    sum the 8 partial vectors, /N, @W2, +b2.
"""

import sys

sys.path.insert(0, "/opt/trn_rl_repo")

from contextlib import ExitStack

import numpy as np

import concourse.tile as tile
from concourse import bacc, bass, mybir
from concourse.bass import IndirectOffsetOnAxis
from concourse.bass_utils import run_bass_kernel_spmd

N = 50000
P = 128
NCORES = 8
TILES = 392            # ceil(N / P) rounded up to a multiple of NCORES
NPAD = TILES * P       # 50176
TPC = TILES // NCORES  # 49 dst tiles per core
NPC = TPC * P          # 6272 nodes per core
D = 128                # feature dim (in & hidden)

F16 = mybir.dt.float16
F32 = mybir.dt.float32
I32 = mybir.dt.int32

_compiled = {}


def _build(C):
    """Build the Bass program. C = max edge K-tiles per dst tile."""
    nc = bacc.Bacc(
        "TRN2", target_bir_lowering=False, debug=False, num_devices=NCORES
    )
    xt = nc.declare_dram_parameter("xt", [P, NPAD], F16, isOutput=False)
    w1 = nc.declare_dram_parameter("w1", [P, D], F16, isOutput=False)
    idx = nc.declare_dram_parameter("idx", [P, TPC * C], I32, isOutput=False)
    ldst = nc.declare_dram_parameter("ldst", [P, TPC * C], F16, isOutput=False)
    disv = nc.declare_dram_parameter("disv", [P, TPC], F32, isOutput=False)
    wv = nc.declare_dram_parameter("wv", [P, TPC], F32, isOutput=False)
    b1b = nc.declare_dram_parameter("b1b", [P, D], F32, isOutput=False)
    sout = nc.declare_dram_parameter("sout", [1, D], F32, isOutput=True)

    hp = nc.dram_tensor("hp", [NPAD, D], F16)  # internal: scaled hidden acts

    with tile.TileContext(nc) as tc, ExitStack() as ctx:
        const = ctx.enter_context(tc.tile_pool(name="const", bufs=1))
        xpool = ctx.enter_context(tc.tile_pool(name="xchunk", bufs=3))
        p1ps = ctx.enter_context(tc.tile_pool(name="p1ps", bufs=2, space="PSUM"))
        hpool = ctx.enter_context(tc.tile_pool(name="hstore", bufs=3))
        gpool = ctx.enter_context(tc.tile_pool(name="gather", bufs=2))
        spool = ctx.enter_context(tc.tile_pool(name="sel", bufs=4))
        aps = ctx.enter_context(tc.tile_pool(name="aggps", bufs=2, space="PSUM"))
        sps = ctx.enter_context(tc.tile_pool(name="sps", bufs=1, space="PSUM"))
        epool = ctx.enter_context(tc.tile_pool(name="epi", bufs=3))
        opool = ctx.enter_context(tc.tile_pool(name="outp", bufs=1))

        # ---- resident constants ----
        w1_sb = const.tile([P, D], F16)
        nc.sync.dma_start(out=w1_sb[:], in_=w1[:])
        # iota row (0..127 along free dim, same on every partition), built
        # on-device so the is_equal consumers only wait on the ldst DMA
        # (DVE TensorTensor encodes at most one sync wait).
        iota_i32 = const.tile([P, D], I32)
        nc.gpsimd.iota(iota_i32[:], pattern=[[1, D]], base=0, channel_multiplier=0)
        iota_sb = const.tile([P, D], F16)
        nc.vector.tensor_copy(out=iota_sb[:], in_=iota_i32[:])
        b1b_sb = const.tile([P, D], F32)
        nc.sync.dma_start(out=b1b_sb[:], in_=b1b[:])
        disv_sb = const.tile([P, TPC], F32)
        nc.sync.dma_start(out=disv_sb[:], in_=disv[:])
        wv_sb = const.tile([P, TPC], F32)
        nc.sync.dma_start(out=wv_sb[:], in_=wv[:])
        idx_sb = const.tile([P, TPC * C], I32)
        nc.sync.dma_start(out=idx_sb[:], in_=idx[:])
        ldst_sb = const.tile([P, TPC * C], F16)
        nc.sync.dma_start(out=ldst_sb[:], in_=ldst[:])

        # ---- phase 1: h' = (dis*x) @ W1 over all NPAD nodes ----
        CH = 2048  # nodes per load chunk
        n0 = 0
        while n0 < NPAD:
            ch = min(CH, NPAD - n0)
            xc = xpool.tile([P, ch], F16, tag="xchunk")
            nc.sync.dma_start(out=xc[:], in_=xt[:, n0 : n0 + ch])
            for g in range(ch // 512):
                ps = p1ps.tile([P, 512], F32)
                for q in range(4):
                    col = g * 512 + q * 128
                    nc.tensor.matmul(
                        out=ps[:, q * 128 : (q + 1) * 128],
                        lhsT=xc[:, col : col + 128],
                        rhs=w1_sb[:],
                        start=True,
                        stop=True,
                    )
                hs = hpool.tile([P, 512], F16, tag="hstore")
                nc.scalar.activation(
                    out=hs[:], in_=ps[:], func=mybir.ActivationFunctionType.Copy
                )
                r0 = n0 + g * 512
                nc.sync.dma_start(
                    out=hp[r0 : r0 + 512, :].rearrange("(q p) f -> p q f", p=P),
                    in_=hs[:].rearrange("p (q f) -> p q f", q=4),
                )
            n0 += ch

        # ---- phase 2: message passing over this core's dst tiles ----
        s_ps = sps.tile([1, D], F32)
        for t in range(TPC):
            gath = gpool.tile([P, C * D], F16, tag="gather")
            nc.gpsimd.indirect_dma_start(
                out=gath[:],
                out_offset=None,
                in_=hp[:],
                in_offset=IndirectOffsetOnAxis(
                    ap=idx_sb[:, t * C : (t + 1) * C], axis=0
                ),
            )
            agg = aps.tile([P, D], F32, tag="agg")
            for c in range(C):
                sel = spool.tile([P, D], F16, tag="sel")
                nc.vector.tensor_tensor(
                    out=sel[:],
                    in0=ldst_sb[:, t * C + c : t * C + c + 1].to_broadcast([P, D]),
                    in1=iota_sb[:],
                    op=mybir.AluOpType.is_equal,
                )
                nc.tensor.matmul(
                    out=agg[:],
                    lhsT=sel[:],
                    rhs=gath[:, c * D : (c + 1) * D],
                    start=(c == 0),
                    stop=(c == C - 1),
                )
            # epilogue: out1 = relu(dis_dst * agg + b1)
            t1 = epool.tile([P, D], F32, tag="t1")
            nc.vector.tensor_scalar_mul(t1[:], agg[:], disv_sb[:, t : t + 1])
            o1 = epool.tile([P, D], F32, tag="o1")
            nc.vector.tensor_tensor(
                out=o1[:], in0=t1[:], in1=b1b_sb[:], op=mybir.AluOpType.add
            )
            nc.scalar.activation(
                out=o1[:], in_=o1[:], func=mybir.ActivationFunctionType.Relu
            )
            # collapsed layer 2: s += w_tile^T @ out1
            nc.tensor.matmul(
                out=s_ps[:],
                lhsT=wv_sb[:, t : t + 1],
                rhs=o1[:],
                start=(t == 0),
                stop=(t == TPC - 1),
                skip_group_check=True,
            )

        s_sb = opool.tile([1, D], F32)
        nc.vector.tensor_copy(out=s_sb[:], in_=s_ps[:])
        nc.sync.dma_start(out=sout[:], in_=s_sb[:])

    nc.compile()
    return nc


def _prep(x, edge_index):
    """Host-side graph preprocessing -> per-core device input maps."""
    src = np.asarray(edge_index[0], dtype=np.int64)
    dst = np.asarray(edge_index[1], dtype=np.int64)
    loop = np.arange(N, dtype=np.int64)
    src_all = np.concatenate([src, loop])
    dst_all = np.concatenate([dst, loop])

    deg = np.bincount(dst_all, minlength=NPAD).astype(np.float64)
    dis = np.zeros(NPAD, dtype=np.float64)
    nz = deg > 0
    dis[nz] = 1.0 / np.sqrt(deg[nz])

    acc = np.zeros(NPAD, dtype=np.float64)
    np.add.at(acc, src_all, dis[dst_all])
    w = dis * acc  # layer-2 collapsed per-node weight

    # bucket edges by dst tile, lay out as [tile, partition, ktile]
    tl = dst_all // P
    order = np.argsort(tl, kind="stable")
    src_s = src_all[order]
    dst_s = dst_all[order]
    tl_s = tl[order]
    counts = np.bincount(tl_s, minlength=TILES)
    C = int(np.ceil(counts.max() / P))
    starts = np.concatenate([[0], np.cumsum(counts)])[:-1]

    j = np.arange(tl_s.size) - starts[tl_s]  # position within tile
    pos = tl_s * (P * C) + (j % P) * C + (j // P)
    idx_flat = np.zeros(TILES * P * C, dtype=np.int32)
    ldst_flat = np.full(TILES * P * C, P, dtype=np.float16)  # sentinel: no match
    idx_flat[pos] = src_s.astype(np.int32)
    ldst_flat[pos] = (dst_s - tl_s * P).astype(np.float16)
    idx_arr = idx_flat.reshape(TILES, P, C)
    ldst_arr = ldst_flat.reshape(TILES, P, C)

    xts = np.zeros((P, NPAD), dtype=np.float16)
    xts[:, :N] = (np.asarray(x, dtype=np.float64) * dis[:N, None]).T

    return C, dis, w, idx_arr, ldst_arr, xts


def kernel(x, edge_index, W1, b1, W2, b2):
    C, dis, w, idx_arr, ldst_arr, xts = _prep(x, edge_index)

    if C not in _compiled:
        _compiled[C] = _build(C)
    nc = _compiled[C]

    w1_d = np.asarray(W1, dtype=np.float16)
    b1b = np.broadcast_to(np.asarray(b1, dtype=np.float32), (P, D)).copy()

    in_maps = []
    for k in range(NCORES):
        t0 = k * TPC
        nbase = t0 * P
        in_maps.append(
            {
                "xt": xts,
                "w1": w1_d,
                "idx": np.ascontiguousarray(
                    idx_arr[t0 : t0 + TPC].transpose(1, 0, 2).reshape(P, TPC * C)
                ),
                "ldst": np.ascontiguousarray(
                    ldst_arr[t0 : t0 + TPC].transpose(1, 0, 2).reshape(P, TPC * C)
                ),
                "disv": np.ascontiguousarray(
                    dis[nbase : nbase + NPC].reshape(TPC, P).T.astype(np.float32)
                ),
                "wv": np.ascontiguousarray(
                    w[nbase : nbase + NPC].reshape(TPC, P).T.astype(np.float32)
                ),
                "b1b": b1b,
            }
        )

    res = run_bass_kernel_spmd(nc, in_maps, core_ids=list(range(NCORES)))
    s_total = np.zeros(D, dtype=np.float64)
    for r in res.results:
        s_total += r["sout"][0].astype(np.float64)

    out = (s_total / N) @ np.asarray(W2, dtype=np.float64) + np.asarray(
        b2, dtype=np.float64
    )
    return out[None, :].astype(np.float32)



# revision 3
# speedup vs baseline: 1.7252x; 1.7252x over previous
"""2-layer GCN (GCNConv -> relu -> GCNConv -> mean) on 8 trn2 NeuronCores.

Math restructure:
  reference output = mean_n(h2[n]) with h2 = A_norm @ (h1 @ W2) + b2,
  h1 = relu(A_norm @ (x @ W1) + b1), A_norm = D^-1/2 (A+I) D^-1/2.
  Since mean is linear and every edge lands on exactly one dst:
    mean(h2) = (1/N) * (sum_n w_n * h1[n]) @ W2 + b2,
    w_n = dis_n * sum_{e: src_e = n} dis_{dst_e}   (edges incl. self-loops)
  so only layer 1 needs real message passing.

Device work per core (SPMD, same program, different data):
  phase 1: h' = (dis * x) @ W1 for ALL nodes (replicated; avoids
           collectives), stored to local DRAM as fp16 rows. Matmul column
           selection is Q-way interleaved so each SBUF partition ends up
           holding Q consecutive hp rows -> contiguous multi-KB store
           descriptors instead of 256B ones.
  phase 2: dst nodes are degree-sorted and assigned to (core, slot,
           partition); tile rank r -> core r%8, slot r//8, so every slot's
           per-node edge capacity C_t (max degree in that slot's 8 tiles)
           is small and the padding overhead stays low. One indirect-DMA
           gather per slot lands each dst node's edge-source rows in its
           own partition; segment-sum is then an in-place pairwise tree of
           DVE adds (no one-hot matmuls at all). Epilogue: *dis_dst, +b1,
           relu, then a [128,1]^T @ [128,128] matmul accumulates the
           w-weighted node sum into a persistent [1,128] PSUM accumulator.
  host:    sum the 8 partial vectors, /N, @W2, +b2.
"""

import sys

sys.path.insert(0, "/opt/trn_rl_repo")

from contextlib import ExitStack

import numpy as np

import concourse.tile as tile
from concourse import bacc, bass, mybir
from concourse.bass import IndirectOffsetOnAxis
from concourse.bass_utils import run_bass_kernel_spmd

N = 50000
P = 128
NCORES = 8
TILES = 392            # ceil(N / P) rounded up to a multiple of NCORES
NPAD = TILES * P       # 50176
TPC = TILES // NCORES  # 49 dst tiles (slots) per core
NPC = TPC * P          # 6272 nodes per core
D = 128                # feature dim (in & hidden)

F16 = mybir.dt.float16
F32 = mybir.dt.float32
I32 = mybir.dt.int32

_compiled = {}


def _build(c_slots):
    """Build the Bass program. c_slots[t] = edge K-capacity of slot t."""
    sumc = int(sum(c_slots))
    cmax = int(max(c_slots))
    nc = bacc.Bacc(
        "TRN2", target_bir_lowering=False, debug=False, num_devices=NCORES
    )
    xt = nc.declare_dram_parameter("xt", [P, NPAD], F16, isOutput=False)
    w1 = nc.declare_dram_parameter("w1", [P, D], F16, isOutput=False)
    idx = nc.declare_dram_parameter("idx", [P, sumc], I32, isOutput=False)
    disv = nc.declare_dram_parameter("disv", [P, TPC], F32, isOutput=False)
    wv = nc.declare_dram_parameter("wv", [P, TPC], F16, isOutput=False)
    b1b = nc.declare_dram_parameter("b1b", [P, D], F32, isOutput=False)
    sout = nc.declare_dram_parameter("sout", [1, D], F32, isOutput=True)

    hp = nc.dram_tensor("hp", [NPAD, D], F16)  # internal: scaled hidden acts

    with tile.TileContext(nc) as tc, ExitStack() as ctx:
        const = ctx.enter_context(tc.tile_pool(name="const", bufs=1))
        xpool = ctx.enter_context(tc.tile_pool(name="xchunk", bufs=3))
        p1ps = ctx.enter_context(tc.tile_pool(name="p1ps", bufs=4, space="PSUM"))
        hpool = ctx.enter_context(tc.tile_pool(name="hstore", bufs=3))
        gpool = ctx.enter_context(tc.tile_pool(name="gather", bufs=3))
        sps = ctx.enter_context(tc.tile_pool(name="sps", bufs=1, space="PSUM"))
        epool = ctx.enter_context(tc.tile_pool(name="epi", bufs=3))
        opool = ctx.enter_context(tc.tile_pool(name="outp", bufs=1))

        # ---- resident constants ----
        w1_sb = const.tile([P, D], F16)
        nc.sync.dma_start(out=w1_sb[:], in_=w1[:])
        b1b_sb = const.tile([P, D], F32)
        nc.sync.dma_start(out=b1b_sb[:], in_=b1b[:])
        disv_sb = const.tile([P, TPC], F32)
        nc.sync.dma_start(out=disv_sb[:], in_=disv[:])
        wv_sb = const.tile([P, TPC], F16)
        nc.sync.dma_start(out=wv_sb[:], in_=wv[:])
        idx_sb = const.tile([P, sumc], I32)
        nc.sync.dma_start(out=idx_sb[:], in_=idx[:])

        # ---- phase 1: h' = (dis*x) @ W1 over all NPAD nodes ----
        CH = 2048  # nodes per chunk == store group
        n0 = 0
        while n0 < NPAD:
            ch = min(CH, NPAD - n0)
            q_iv = ch // P  # row interleave factor (16 or 8)
            xc = xpool.tile([P, CH], F16, tag="xchunk")
            nc.sync.dma_start(out=xc[:, :ch], in_=xt[:, n0 : n0 + ch])
            # 3D view: xv[k, j, p] = x feature k of node n0 + p*q_iv + j
            xv = xc[:, :ch].rearrange("k (p q) -> k q p", q=q_iv)
            hs = hpool.tile([P, CH], F16, tag="hstore")
            for g in range(ch // 512):
                ps = p1ps.tile([P, 512], F32)
                for q in range(4):
                    j = g * 4 + q
                    nc.tensor.matmul(
                        out=ps[:, q * 128 : (q + 1) * 128],
                        lhsT=xv[:, j, :],
                        rhs=w1_sb[:],
                        start=True,
                        stop=True,
                    )
                nc.scalar.activation(
                    out=hs[:, g * 512 : (g + 1) * 512],
                    in_=ps[:],
                    func=mybir.ActivationFunctionType.Copy,
                )
            # partition p holds nodes n0 + p*q_iv .. n0 + p*q_iv + q_iv-1
            # in order -> per-partition contiguous q_iv*256B descriptor.
            nc.sync.dma_start(
                out=hp[n0 : n0 + ch, :].rearrange("(p q) f -> p (q f)", q=q_iv),
                in_=hs[:, :ch],
            )
            n0 += ch

        # ---- phase 2: gather + per-partition segment sum per slot ----
        s_ps = sps.tile([1, D], F32)
        off = 0
        for t in range(TPC):
            c = int(c_slots[t])
            gath = gpool.tile([P, cmax * D], F16, tag="gather")
            nc.gpsimd.indirect_dma_start(
                out=gath[:, : c * D],
                out_offset=None,
                in_=hp[:],
                in_offset=IndirectOffsetOnAxis(
                    ap=idx_sb[:, off : off + c], axis=0
                ),
            )
            # in-place pairwise tree: fold top half onto bottom half
            cur = c
            while cur > 1:
                h = cur // 2
                r = cur - h
                nc.vector.tensor_tensor(
                    out=gath[:, : h * D],
                    in0=gath[:, : h * D],
                    in1=gath[:, r * D : cur * D],
                    op=mybir.AluOpType.add,
                )
                cur = r
            # epilogue: o1 = relu(dis_dst * agg + b1)  (fp16 out)
            t1 = epool.tile([P, D], F32, tag="t1")
            nc.vector.tensor_scalar_mul(
                t1[:], gath[:, :D], disv_sb[:, t : t + 1]
            )
            nc.vector.tensor_tensor(
                out=t1[:], in0=t1[:], in1=b1b_sb[:], op=mybir.AluOpType.add
            )
            o1 = epool.tile([P, D], F16, tag="o1")
            nc.scalar.activation(
                out=o1[:], in_=t1[:], func=mybir.ActivationFunctionType.Relu
            )
            # collapsed layer 2: s += w_slot^T @ o1
            nc.tensor.matmul(
                out=s_ps[:],
                lhsT=wv_sb[:, t : t + 1],
                rhs=o1[:],
                start=(t == 0),
                stop=(t == TPC - 1),
                skip_group_check=True,
            )
            off += c

        s_sb = opool.tile([1, D], F32)
        nc.vector.tensor_copy(out=s_sb[:], in_=s_ps[:])
        nc.sync.dma_start(out=sout[:], in_=s_sb[:])

    nc.compile()
    return nc


def _prep(x, edge_index):
    """Host-side graph preprocessing -> per-core device input maps."""
    src = np.asarray(edge_index[0], dtype=np.int64)
    dst = np.asarray(edge_index[1], dtype=np.int64)
    loop = np.arange(N, dtype=np.int64)
    src_all = np.concatenate([src, loop])
    dst_all = np.concatenate([dst, loop])

    deg = np.bincount(dst_all, minlength=NPAD).astype(np.int64)
    dis = np.zeros(NPAD, dtype=np.float64)
    nz = deg > 0
    dis[nz] = 1.0 / np.sqrt(deg[nz])

    acc = np.zeros(NPAD, dtype=np.float64)
    np.add.at(acc, src_all, dis[dst_all])
    w = dis * acc  # layer-2 collapsed per-node weight

    # degree-sorted relabeling: rank r -> node perm[r];
    # tile rank rt = r // P -> core rt % 8, slot rt // 8, partition r % P.
    perm = np.argsort(-deg, kind="stable")
    rank = np.empty(NPAD, dtype=np.int64)
    rank[perm] = np.arange(NPAD)
    degs = deg[perm]
    c_slots = tuple(
        int(max(1, degs[(NCORES * t) * P])) for t in range(TPC)
    )
    offs = np.concatenate([[0], np.cumsum(c_slots)]).astype(np.int64)
    sumc = int(offs[-1])

    # per-dst contiguous edge runs
    order = np.argsort(dst_all, kind="stable")
    src_s = src_all[order].astype(np.int32)
    dst_s = dst_all[order]
    starts = np.concatenate([[0], np.cumsum(np.bincount(dst_all, minlength=NPAD))])
    j = np.arange(dst_s.size, dtype=np.int64) - starts[dst_s]

    r = rank[dst_s]
    rt = r // P
    core = rt % NCORES
    slot = rt // NCORES
    p = r % P
    col = offs[slot] + j

    idx_full = np.full((NCORES, P, sumc), N, dtype=np.int32)  # pad -> zero row
    idx_full[core, p, col] = src_s

    # per-core dis / w vectors in (partition, slot) layout
    node_of = perm.reshape(TILES, P)  # [tile rank, partition] -> node
    disv_full = np.empty((NCORES, P, TPC), dtype=np.float32)
    wv_full = np.empty((NCORES, P, TPC), dtype=np.float16)
    for k in range(NCORES):
        sel = node_of[k::NCORES, :]  # [TPC, P]
        disv_full[k] = dis[sel].T.astype(np.float32)
        wv_full[k] = w[sel].T.astype(np.float16)

    xts = np.zeros((P, NPAD), dtype=np.float16)
    xts[:, :N] = (np.asarray(x, dtype=np.float64) * dis[:N, None]).T

    return c_slots, idx_full, disv_full, wv_full, xts


def _make_in_maps(inputs):
    c_slots, idx_full, disv_full, wv_full, xts = _prep(
        inputs["x"], inputs["edge_index"]
    )
    w1_d = np.asarray(inputs["W1"], dtype=np.float16)
    b1b = np.broadcast_to(
        np.asarray(inputs["b1"], dtype=np.float32), (P, D)
    ).copy()
    in_maps = []
    for k in range(NCORES):
        in_maps.append(
            {
                "xt": xts,
                "w1": w1_d,
                "idx": np.ascontiguousarray(idx_full[k]),
                "disv": np.ascontiguousarray(disv_full[k]),
                "wv": np.ascontiguousarray(wv_full[k]),
                "b1b": b1b,
            }
        )
    return c_slots, in_maps


def _run(inputs, trace=False):
    c_slots, in_maps = _make_in_maps(inputs)
    if c_slots not in _compiled:
        _compiled[c_slots] = _build(c_slots)
    nc = _compiled[c_slots]

    res = run_bass_kernel_spmd(
        nc, in_maps, core_ids=list(range(NCORES)), trace=trace
    )
    s_total = np.zeros(D, dtype=np.float64)
    for r in res.results:
        s_total += r["sout"][0].astype(np.float64)

    out = (s_total / N) @ np.asarray(inputs["W2"], dtype=np.float64) + np.asarray(
        inputs["b2"], dtype=np.float64
    )
    return out[None, :].astype(np.float32), res.exec_time_ns


def kernel(x, edge_index, W1, b1, W2, b2):
    out, _ = _run(
        {
            "x": x,
            "edge_index": edge_index,
            "W1": W1,
            "b1": b1,
            "W2": W2,
            "b2": b2,
        }
    )
    return out
